# revision 11
# baseline (speedup 1.0000x reference)
"""NeuroSAT message-passing kernel for 8 Trainium2 NeuronCores.

Sharding (per spec hint): literals row-sharded (500/core, padded 512),
clauses column-sharded (1000/core, padded 1024). Each step:
  X = mlp_LC(Lh) locally -> AllGather X -> LC_msgs = Mcol.T @ X (local GEMM)
  Y = mlp_CL(Ch) locally -> AllGather Y -> CL_msgs = Mrow @ Y (local GEMM)
  LSTM updates sharded row-wise.

Precision: activations/weights fp16 into the PE (fp32 PSUM accumulate,
fp32 elementwise); M shards are fp8e4 (binary 0/1 -> exact). Validated
~1e-3 relative error on the final logit.

Schedule: software-pipelined one step ahead — the MLPs + AllGathers for
step t+1 are interleaved with the dir-GEMMs/LSTMs of step t so the
collectives and gather-buffer DMA loads hide under PE work. The X-side
(AllGather consumed soonest) is further split into two half-row
collectives so dir1 can start on the first half while the second flies.

Layout: activations/states are stored transposed [feature(=partition), row].
M shards are host-prepared in the exact SBUF layouts:
  A [128, 32, 1024]: A[p, o, n] = Mpad[128*o + p, clause_shard_n]  (lit-contraction)
  B [128, 64, 512]:  B[p, o, n] = Mpad[lit_shard_n, 128*o + p]     (clause-contraction)
where Mpad is [4096, 8192] with each rank's 500 lits / 1000 clauses zero-padded
to 512 / 1024 (matching AllGather's rank-concatenated layout).
"""

import os
import sys

sys.path.insert(0, "/opt/trn_rl_repo")

import numpy as np

R = 8
EMBED = 128
NLR, NCR = 500, 1000   # real rows per core
NLP, NCP = 512, 1024   # padded rows per core
T_STEPS = int(os.environ.get("NSAT_STEPS", "30"))

LAST_EXEC_NS = None
LAST_RESULTS = None

_program_cache = {}


def build_program(T=T_STEPS):
    import concourse.bass as bass  # noqa: F401
    import concourse.mybir as mybir
    import concourse.tile as tile
    from concourse import bacc
    from concourse.masks import make_identity
    from contextlib import ExitStack

    dt = mybir.dt
    f16, f32, f8 = dt.float16, dt.float32, dt.float8e4
    AO = mybir.AluOpType
    AF = mybir.ActivationFunctionType

    nc = bacc.Bacc(
        "TRN2",
        target_bir_lowering=False,
        debug=False,
        enable_asserts=False,
        num_devices=R,
    )

    def din(name, shape, d=f16):
        return nc.dram_tensor(name, list(shape), d, kind="ExternalInput").ap()

    A_d = din("A", (128, 32, NCP), f8)
    B_d = din("B", (128, 64, NLP), f8)
    prm = {}
    for side in ("lc", "cl", "v"):
        wout = 1 if side == "v" else 128
        prm[side + "_w1t"] = din(side + "_w1t", (128, 400))
        prm[side + "_w2t"] = din(side + "_w2t", (128, 3, 200))
        prm[side + "_w2t_r"] = din(side + "_w2t_r", (16, 200))
        prm[side + "_w3t"] = din(side + "_w3t", (128, wout))
        prm[side + "_w3t_r"] = din(side + "_w3t_r", (72, wout))
        prm[side + "_b1"] = din(side + "_b1", (128, 4), f32)
        prm[side + "_b2"] = din(side + "_b2", (128, 2), f32)
        prm[side + "_b3"] = din(side + "_b3", (wout, 1), f32)
    for side in ("l", "c"):
        prm[side + "_wiht"] = din(side + "_wiht", (128, 512))
        prm[side + "_whht"] = din(side + "_whht", (128, 512))
        prm[side + "_b"] = din(side + "_b", (128, 4), f32)
    lh0_d = din("lh0", (128, NLP))
    ch0_d = din("ch0", (128, NCP))
    votes_d = nc.dram_tensor("votes", [1, NLP], f32, kind="ExternalOutput").ap()

    with tile.TileContext(nc) as tc, ExitStack() as ctx:
        st = ctx.enter_context(tc.tile_pool(name="static", bufs=1))
        h1p = ctx.enter_context(tc.tile_pool(name="h1p", bufs=2))
        h2p = ctx.enter_context(tc.tile_pool(name="h2p", bufs=2))
        xyp = ctx.enter_context(tc.tile_pool(name="xyp", bufs=3))
        stp = ctx.enter_context(tc.tile_pool(name="stp", bufs=2))
        msgp = ctx.enter_context(tc.tile_pool(name="msgp", bufs=2))
        tmpp = ctx.enter_context(tc.tile_pool(name="tmpp", bufs=4))
        natp = ctx.enter_context(tc.tile_pool(name="natp", bufs=2))
        dramp = ctx.enter_context(tc.tile_pool(name="dramp", bufs=2, space="DRAM"))
        ps_net = ctx.enter_context(tc.tile_pool(name="ps_net", bufs=2, space="PSUM"))
        ps_gate = ctx.enter_context(tc.tile_pool(name="ps_gate", bufs=2, space="PSUM"))
        ps_dir = ctx.enter_context(tc.tile_pool(name="ps_dir", bufs=2, space="PSUM"))
        ps_tr = ctx.enter_context(tc.tile_pool(name="ps_tr", bufs=2, space="PSUM"))

        # ---- load static data (M shards on gpsimd queues, rest on sync) ----
        W = {}
        for k, ap in prm.items():
            W[k] = st.tile(list(ap.shape), ap.dtype, name=k + "_s")
            nc.sync.dma_start(W[k][:], ap[:])
        lh = st.tile([128, NLP], f16, name="lh_s")
        ch = st.tile([128, NCP], f16, name="ch_s")
        nc.sync.dma_start(lh[:], lh0_d[:])
        nc.sync.dma_start(ch[:], ch0_d[:])
        lcst = st.tile([128, NLP], f32, name="lc_st")
        ccst = st.tile([128, NCP], f32, name="cc_st")
        nc.vector.memset(lcst[:], 0.0)
        nc.vector.memset(ccst[:], 0.0)
        ident = st.tile([128, 128], f16, name="ident_s")
        make_identity(nc, ident[:])
        A_s = st.tile([128, 32, NCP], f8, name="A_s")
        B_s = st.tile([128, 64, NLP], f8, name="B_s")
        for q in range(8):
            nc.gpsimd.dma_start(
                A_s[:, 4 * q : 4 * q + 4, :], A_d[:, 4 * q : 4 * q + 4, :]
            )
            nc.gpsimd.dma_start(
                B_s[:, 8 * q : 8 * q + 8, :], B_d[:, 8 * q : 8 * q + 8, :]
            )

        replica_groups = [list(range(R))]

        def mlp_chunk(pre, rhs, out_tile, out_parts, ncols):
            """3-layer MLP on a [128, ncols] fp16 activation chunk (transposed
            layout). Writes (+bias, no final relu) into out_tile[:out_parts, :ncols]."""
            w1t, w2t, w2t_r = W[pre + "_w1t"], W[pre + "_w2t"], W[pre + "_w2t_r"]
            w3t, w3t_r = W[pre + "_w3t"], W[pre + "_w3t_r"]
            b1, b2, b3 = W[pre + "_b1"], W[pre + "_b2"], W[pre + "_b3"]
            h1 = h1p.tile([128, 4, ncols], f16, tag="h1")
            for c in range(4):
                m = 128 if c < 3 else 16
                ps = ps_net.tile([128, 512], f32, tag="psnet")
                nc.tensor.matmul(
                    ps[:m, :ncols], w1t[:, 128 * c : 128 * c + m], rhs,
                    start=True, stop=True,
                )
                nc.vector.tensor_scalar(
                    h1[:m, c, :], ps[:m, :ncols], b1[:m, c : c + 1], 0.0,
                    AO.add, AO.max,
                )
            h2 = h2p.tile([128, 2, ncols], f16, tag="h2")
            for c, m in ((0, 128), (1, 72)):
                ps = ps_net.tile([128, 512], f32, tag="psnet")
                for k in range(3):
                    nc.tensor.matmul(
                        ps[:m, :ncols],
                        w2t[:, k, 128 * c : 128 * c + m],
                        h1[:, k, :],
                        start=(k == 0),
                        stop=False,
                    )
                nc.tensor.matmul(
                    ps[:m, :ncols],
                    w2t_r[:, 128 * c : 128 * c + m],
                    h1[0:16, 3, :],
                    start=False,
                    stop=True,
                )
                nc.vector.tensor_scalar(
                    h2[:m, c, :], ps[:m, :ncols], b2[:m, c : c + 1], 0.0,
                    AO.add, AO.max,
                )
            o = out_parts
            ps = ps_net.tile([128, 512], f32, tag="psnet")
            nc.tensor.matmul(
                ps[:o, :ncols], w3t[:, :o], h2[:, 0, :], start=True, stop=False
            )
            nc.tensor.matmul(
                ps[:o, :ncols], w3t_r[:, :o], h2[0:72, 1, :], start=False, stop=True
            )
            nc.vector.tensor_scalar(
                out_tile[:o, :ncols], ps[:o, :ncols], b3[:o, 0:1], None, AO.add
            )

        def transpose_to_stage(srcT, ncols, stage, blk0):
            for cb in range(ncols // 128):
                pst = ps_tr.tile([128, 128], f16, tag="pstr")
                nc.tensor.transpose(
                    pst[:], srcT[:, 128 * cb : 128 * (cb + 1)], ident[:]
                )
                nc.vector.tensor_copy(out=stage[:, blk0 + cb, :], in_=pst[:])

        def gate(wih, whh, b, g, msg, h_sl, func, ncols):
            ps = ps_gate.tile([128, 512], f32, tag="gate")
            nc.tensor.matmul(
                ps[:, :ncols], wih[:, 128 * g : 128 * (g + 1)], msg,
                start=True, stop=False,
            )
            nc.tensor.matmul(
                ps[:, :ncols], whh[:, 128 * g : 128 * (g + 1)], h_sl,
                start=False, stop=True,
            )
            t = tmpp.tile([128, ncols], f32, tag="tmp")
            nc.scalar.activation(t[:], ps[:, :ncols], func, bias=b[:, g : g + 1])
            return t

        def lstm_chunk(pre, msg, h_sl, c_sl, ncols):
            """gates order i,f,g,o. h_sl ([128,ncols] f16) and c_sl (f32) updated
            in place."""
            wih, whh, b = W[pre + "_wiht"], W[pre + "_whht"], W[pre + "_b"]
            tf = gate(wih, whh, b, 1, msg, h_sl, AF.Sigmoid, ncols)
            nc.vector.tensor_mul(out=c_sl, in0=tf[:], in1=c_sl)
            ti = gate(wih, whh, b, 0, msg, h_sl, AF.Sigmoid, ncols)
            tg = gate(wih, whh, b, 2, msg, h_sl, AF.Tanh, ncols)
            nc.vector.tensor_mul(out=ti[:], in0=ti[:], in1=tg[:])
            nc.vector.tensor_add(out=c_sl, in0=c_sl, in1=ti[:])
            tc_ = tmpp.tile([128, ncols], f32, tag="tmp")
            nc.scalar.activation(tc_[:], c_sl, AF.Tanh)
            to = gate(wih, whh, b, 3, msg, h_sl, AF.Sigmoid, ncols)
            nc.vector.tensor_mul(out=h_sl, in0=to[:], in1=tc_[:])

        def emit_y_side(ynat_dst):
            """MLP_CL on current ch -> Y -> AllGather -> load into ynat_dst."""
            with nc.named_scope("mlp_cl"):
                yin_t = dramp.tile([NCP, 128], f16, tag="yin")
                yin_v = yin_t.rearrange("(o p) e -> p o e", p=128)
                for n in range(2):
                    yT = xyp.tile([128, 512], f16, tag="xyT")
                    mlp_chunk("cl", ch[:, 512 * n : 512 * (n + 1)], yT, 128, 512)
                    ystage = stp.tile([128, 4, 128], f16, tag="stage")
                    transpose_to_stage(yT, 512, ystage, 0)
                    nc.sync.dma_start(yin_v[:, 4 * n : 4 * n + 4, :], ystage[:, :, :])
            yout_t = dramp.tile([NCP * R, 128], f16, tag="yout", addr_space="Shared")
            nc.gpsimd.collective_compute(
                "AllGather", AO.bypass,
                ins=[yin_t.opt()], outs=[yout_t.opt()],
                replica_groups=replica_groups,
            )
            yv = yout_t.rearrange("(r j p) e -> p r j e", p=128, j=8)
            for r in range(R):
                nc.gpsimd.dma_start(ynat_dst[:, 8 * r : 8 * r + 8, :], yv[:, r])

        def emit_x_half(h, xnat_dst):
            """MLP_LC on lh half h -> X half -> AllGather -> load into xnat_dst."""
            with nc.named_scope("mlp_lc"):
                xT = xyp.tile([128, 256], f16, tag="xyT")
                mlp_chunk("lc", lh[:, 256 * h : 256 * (h + 1)], xT, 128, 256)
                xstage = stp.tile([128, 2, 128], f16, tag="xstage")
                transpose_to_stage(xT, 256, xstage, 0)
                xin_t = dramp.tile([NLP // 2, 128], f16, tag=f"xin{h}")
                nc.sync.dma_start(
                    xin_t.rearrange("(o p) e -> p o e", p=128), xstage[:, :, :]
                )
            xout_t = dramp.tile(
                [NLP // 2 * R, 128], f16, tag=f"xout{h}", addr_space="Shared"
            )
            nc.gpsimd.collective_compute(
                "AllGather", AO.bypass,
                ins=[xin_t.opt()], outs=[xout_t.opt()],
                replica_groups=replica_groups,
            )
            # xnat slot order: [h, r, j] — halves contiguous; A is host-permuted
            # to the same row-chunk order.
            xv = xout_t.rearrange("(q p) e -> p q e", p=128)
            for hq in range(2):
                nc.gpsimd.dma_start(
                    xnat_dst[:, 16 * h + 8 * hq : 16 * h + 8 * hq + 8, :],
                    xv[:, 8 * hq : 8 * hq + 8, :],
                )

        # ---- prologue: X(0), Y(0) ----
        xnat_next = natp.tile([128, 32, 128], f16, tag="xnat")
        ynat_next = natp.tile([128, 64, 128], f16, tag="ynat")
        for h in range(2):
            emit_x_half(h, xnat_next)
        emit_y_side(ynat_next)

        for t in range(T):
            xnat_cur, ynat_cur = xnat_next, ynat_next

            # --- dir1(t): LC_msgs^T; LSTM_C(t) ---
            for n in range(2):
                with nc.named_scope("dir1"):
                    psd = ps_dir.tile([128, 512], f32, tag="dir")
                    for k in range(32):
                        nc.tensor.matmul(
                            psd[:],
                            xnat_cur[:, k, :],
                            A_s[:, k, 512 * n : 512 * (n + 1)],
                            start=(k == 0),
                            stop=(k == 31),
                        )
                    msg = msgp.tile([128, 512], f16, tag="msg")
                    nc.vector.tensor_copy(out=msg[:], in_=psd[:])
                with nc.named_scope("lstm_c"):
                    lstm_chunk(
                        "c", msg[:],
                        ch[:, 512 * n : 512 * (n + 1)],
                        ccst[:, 512 * n : 512 * (n + 1)],
                        512,
                    )

            # --- dir2(t) in row halves; LSTM_L(t); MLP_LC(t+1) + AGX(t+1) ---
            if t + 1 < T:
                xnat_next = natp.tile([128, 32, 128], f16, tag="xnat")
            for h in range(2):
                with nc.named_scope("dir2"):
                    psd2 = ps_dir.tile([128, 512], f32, tag="dir")
                    for k in range(64):
                        nc.tensor.matmul(
                            psd2[:, :256],
                            ynat_cur[:, k, :],
                            B_s[:, k, 256 * h : 256 * (h + 1)],
                            start=(k == 0),
                            stop=(k == 63),
                        )
                    msgl = msgp.tile([128, 256], f16, tag="msg")
                    nc.vector.tensor_copy(out=msgl[:], in_=psd2[:, :256])
                with nc.named_scope("lstm_l"):
                    lstm_chunk(
                        "l", msgl[:],
                        lh[:, 256 * h : 256 * (h + 1)],
                        lcst[:, 256 * h : 256 * (h + 1)],
                        256,
                    )
                if t + 1 < T:
                    emit_x_half(h, xnat_next)

            # --- MLP_CL(t+1) + AGY(t+1): PE filler while AGX(t+1) flies;
            #     AGY runs on CC after the AGX halves, during dir1(t+1) ---
            if t + 1 < T:
                ynat_next = natp.tile([128, 64, 128], f16, tag="ynat")
                emit_y_side(ynat_next)

        # --- final: vote MLP on Lh ---
        votes_sb = st.tile([1, 512], f32, name="votes_sb")
        mlp_chunk("v", lh[:, :], votes_sb, 1, 512)
        nc.sync.dma_start(votes_d[:], votes_sb[0:1, :])

    nc.compile()
    return nc


def prep_inputs(inputs):
    """Host-side: shard + lay out all tensors per core in final dtypes."""
    import ml_dtypes

    f8 = ml_dtypes.float8_e4m3
    f16 = np.float16
    f32 = np.float32
    M = np.asarray(inputs["M"], f32)

    MP = np.zeros((NLP * R, NCP * R), f32)
    for rl in range(R):
        for rc in range(R):
            MP[NLP * rl : NLP * rl + NLR, NCP * rc : NCP * rc + NCR] = M[
                NLR * rl : NLR * (rl + 1), NCR * rc : NCR * (rc + 1)
            ]

    shared = {}

    def mlp_prep(pre, W1, b1, W2, b2, W3, b3):
        wout = W3.shape[0]
        shared[pre + "_w1t"] = np.ascontiguousarray(np.asarray(W1, f32).T, f16)
        w2t_full = np.zeros((512, 200), f32)
        w2t_full[:400] = np.asarray(W2, f32).T
        shared[pre + "_w2t"] = np.ascontiguousarray(
            w2t_full[:384].reshape(3, 128, 200).transpose(1, 0, 2), f16
        )
        shared[pre + "_w2t_r"] = np.ascontiguousarray(w2t_full[384:400], f16)
        w3t = np.ascontiguousarray(np.asarray(W3, f32).T, f32)   # [200, wout]
        shared[pre + "_w3t"] = np.ascontiguousarray(w3t[:128], f16)
        shared[pre + "_w3t_r"] = np.ascontiguousarray(w3t[128:200], f16)
        b1p = np.zeros((512,), f32); b1p[:400] = b1
        shared[pre + "_b1"] = np.ascontiguousarray(b1p.reshape(4, 128).T, f32)
        b2p = np.zeros((256,), f32); b2p[:200] = b2
        shared[pre + "_b2"] = np.ascontiguousarray(b2p.reshape(2, 128).T, f32)
        b3p = np.zeros((wout, 1), f32); b3p[:, 0] = b3
        shared[pre + "_b3"] = b3p

    mlp_prep("lc", inputs["LC_W1"], inputs["LC_b1"], inputs["LC_W2"],
             inputs["LC_b2"], inputs["LC_W3"], inputs["LC_b3"])
    mlp_prep("cl", inputs["CL_W1"], inputs["CL_b1"], inputs["CL_W2"],
             inputs["CL_b2"], inputs["CL_W3"], inputs["CL_b3"])
    mlp_prep("v", inputs["V_W1"], inputs["V_b1"], inputs["V_W2"],
             inputs["V_b2"], inputs["V_W3"], inputs["V_b3"])

    for pre, Wih, Whh, b in (
        ("l", inputs["L_Wih"], inputs["L_Whh"], inputs["L_b"]),
        ("c", inputs["C_Wih"], inputs["C_Whh"], inputs["C_b"]),
    ):
        shared[pre + "_wiht"] = np.ascontiguousarray(np.asarray(Wih, f32).T, f16)
        shared[pre + "_whht"] = np.ascontiguousarray(np.asarray(Whh, f32).T, f16)
        shared[pre + "_b"] = np.ascontiguousarray(
            np.asarray(b, f32).reshape(4, 128).T, f32
        )

    shared["lh0"] = np.ascontiguousarray(
        np.repeat(np.asarray(inputs["L_init"], f32)[:, None], NLP, 1), f16
    )
    shared["ch0"] = np.ascontiguousarray(
        np.repeat(np.asarray(inputs["C_init"], f32)[:, None], NCP, 1), f16
    )

    in_maps = []
    for r in range(R):
        A_r = MP[:, NCP * r : NCP * (r + 1)]                       # [4096, 1024]
        B_r = MP[NLP * r : NLP * (r + 1), :].T                     # [8192, 512]
        m = dict(shared)
        # row-chunk order must match xnat slot order o' = 16h + 2r + j
        # (lit rows for slot o' are 512r + 256h + 128j .. +128)
        A_chunks = A_r.reshape(32, 128, NCP)
        perm = [4 * rr + 2 * hh + jj
                for hh in range(2) for rr in range(8) for jj in range(2)]
        m["A"] = np.ascontiguousarray(
            A_chunks[perm].transpose(1, 0, 2), f8
        )
        m["B"] = np.ascontiguousarray(
            B_r.reshape(64, 128, NLP).transpose(1, 0, 2), f8
        )
        in_maps.append(m)
    return in_maps


def _install_ntff_hook():
    """The image's antenv package lacks axon_hooks; shim it so
    run_bass_kernel_spmd(trace=True) can capture NTFF profiles."""
    import types

    if "antenv.axon_hooks" in sys.modules:
        return
    if "/root/.axon_site" not in sys.path:
        sys.path.insert(0, "/root/.axon_site")
    try:
        from trn_agent_boot.trn_boot import _ntff_profile_via_ctypes

        hook = _ntff_profile_via_ctypes("/opt/axon/libaxon_pjrt.so")
    except Exception:
        hook = None
    mod = types.ModuleType("antenv.axon_hooks")
    state = {"hook": hook}
    mod.set_axon_ntff_profile_hook = lambda h: state.__setitem__("hook", h)
    mod.get_axon_ntff_profile_hook = lambda: state["hook"]
    sys.modules["antenv.axon_hooks"] = mod
    try:
        import antenv

        antenv.axon_hooks = mod
    except Exception:
        pass


def kernel(**inputs):
    global LAST_EXEC_NS, LAST_RESULTS
    from concourse.bass_utils import run_bass_kernel_spmd

    T = T_STEPS
    if T not in _program_cache:
        _program_cache[T] = build_program(T)
    nc = _program_cache[T]

    in_maps = prep_inputs(inputs)
    trace = os.environ.get("NSAT_TRACE", "0") == "1"
    if trace:
        _install_ntff_hook()
    res = run_bass_kernel_spmd(nc, in_maps, list(range(R)), trace=trace)
    LAST_EXEC_NS = res.exec_time_ns
    LAST_RESULTS = res
    votes_pre = np.concatenate(
        [res.results[r]["votes"][0, :NLR].astype(np.float64) for r in range(R)]
    )
    avg = np.mean(1.0 / (1.0 + np.exp(-votes_pre)))
    return np.asarray(np.log(avg / (1.0 - avg)), dtype=np.float32)


# revision 12
# speedup vs baseline: 1.1060x; 1.1060x over previous
"""NeuroSAT message-passing kernel for 8 Trainium2 NeuronCores.

Sharding (per spec hint): literals row-sharded (500/core, padded 512),
clauses column-sharded (1000/core, padded 1024). Each step:
  X = mlp_LC(Lh) locally -> AllGather X -> LC_msgs = Mcol.T @ X (local GEMM)
  Y = mlp_CL(Ch) locally -> AllGather Y -> CL_msgs = Mrow @ Y (local GEMM)
  LSTM updates sharded row-wise.

Precision: activations/weights fp16 into the PE (fp32 PSUM accumulate,
fp32 elementwise); M shards are fp8e4 (binary 0/1 -> exact). Validated
~1e-3 relative error on the final logit.

Schedule: software-pipelined one step ahead — the MLPs + AllGathers for
step t+1 are interleaved with the dir-GEMMs/LSTMs of step t so the
collectives and gather-buffer DMA loads hide under PE work. The X-side
(AllGather consumed soonest) is further split into two half-row
collectives so dir1 can start on the first half while the second flies.

Layout: activations/states are stored transposed [feature(=partition), row].
M shards are host-prepared in the exact SBUF layouts:
  A [128, 32, 1024]: A[p, o, n] = Mpad[128*o + p, clause_shard_n]  (lit-contraction)
  B [128, 64, 512]:  B[p, o, n] = Mpad[lit_shard_n, 128*o + p]     (clause-contraction)
where Mpad is [4096, 8192] with each rank's 500 lits / 1000 clauses zero-padded
to 512 / 1024 (matching AllGather's rank-concatenated layout).
"""

import os
import sys

sys.path.insert(0, "/opt/trn_rl_repo")

import numpy as np

R = 8
EMBED = 128
NLR, NCR = 500, 1000   # real rows per core
NLP, NCP = 512, 1024   # padded rows per core
T_STEPS = int(os.environ.get("NSAT_STEPS", "30"))

LAST_EXEC_NS = None
LAST_RESULTS = None

_program_cache = {}


def build_program(T=T_STEPS):
    import concourse.bass as bass  # noqa: F401
    import concourse.mybir as mybir
    import concourse.tile as tile
    from concourse import bacc
    from concourse.masks import make_identity
    from contextlib import ExitStack

    dt = mybir.dt
    f16, f32, f8 = dt.float16, dt.float32, dt.float8e4
    AO = mybir.AluOpType
    AF = mybir.ActivationFunctionType

    nc = bacc.Bacc(
        "TRN2",
        target_bir_lowering=False,
        debug=False,
        enable_asserts=False,
        num_devices=R,
    )

    def din(name, shape, d=f16):
        return nc.dram_tensor(name, list(shape), d, kind="ExternalInput").ap()

    A_d = din("A", (128, 32, NCP), f8)
    B_d = din("B", (128, 64, NLP), f8)
    prm = {}
    for side in ("lc", "cl", "v"):
        wout = 1 if side == "v" else 128
        prm[side + "_w1t"] = din(side + "_w1t", (128, 400))
        prm[side + "_w2t"] = din(side + "_w2t", (128, 3, 200))
        prm[side + "_w2t_r"] = din(side + "_w2t_r", (16, 200))
        prm[side + "_w3t"] = din(side + "_w3t", (128, wout))
        prm[side + "_w3t_r"] = din(side + "_w3t_r", (72, wout))
        prm[side + "_b1"] = din(side + "_b1", (128, 4), f32)
        prm[side + "_b2"] = din(side + "_b2", (128, 2), f32)
        prm[side + "_b3"] = din(side + "_b3", (wout, 1), f32)
    for side in ("l", "c"):
        prm[side + "_wiht"] = din(side + "_wiht", (128, 512))
        prm[side + "_whht"] = din(side + "_whht", (128, 512))
        prm[side + "_b"] = din(side + "_b", (128, 4), f32)
    lh0_d = din("lh0", (128, NLP))
    ch0_d = din("ch0", (128, NCP))
    votes_d = nc.dram_tensor("votes", [1, NLP], f32, kind="ExternalOutput").ap()

    with tile.TileContext(nc) as tc, ExitStack() as ctx:
        st = ctx.enter_context(tc.tile_pool(name="static", bufs=1))
        h1p = ctx.enter_context(tc.tile_pool(name="h1p", bufs=2))
        h2p = ctx.enter_context(tc.tile_pool(name="h2p", bufs=2))
        xyp = ctx.enter_context(tc.tile_pool(name="xyp", bufs=3))
        stp = ctx.enter_context(tc.tile_pool(name="stp", bufs=2))
        msgp = ctx.enter_context(tc.tile_pool(name="msgp", bufs=2))
        tmpp = ctx.enter_context(tc.tile_pool(name="tmpp", bufs=4))
        natp = ctx.enter_context(tc.tile_pool(name="natp", bufs=2))
        dramp = ctx.enter_context(tc.tile_pool(name="dramp", bufs=2, space="DRAM"))
        ps_net = ctx.enter_context(tc.tile_pool(name="ps_net", bufs=2, space="PSUM"))
        ps_gate = ctx.enter_context(tc.tile_pool(name="ps_gate", bufs=2, space="PSUM"))
        ps_dir = ctx.enter_context(tc.tile_pool(name="ps_dir", bufs=2, space="PSUM"))
        ps_tr = ctx.enter_context(tc.tile_pool(name="ps_tr", bufs=2, space="PSUM"))

        # ---- load static data (M shards on gpsimd queues, rest on sync) ----
        W = {}
        for k, ap in prm.items():
            W[k] = st.tile(list(ap.shape), ap.dtype, name=k + "_s")
            nc.sync.dma_start(W[k][:], ap[:])
        lh = st.tile([128, NLP], f16, name="lh_s")
        ch = st.tile([128, NCP], f16, name="ch_s")
        nc.sync.dma_start(lh[:], lh0_d[:])
        nc.sync.dma_start(ch[:], ch0_d[:])
        lcst = st.tile([128, NLP], f32, name="lc_st")
        ccst = st.tile([128, NCP], f32, name="cc_st")
        nc.vector.memset(lcst[:], 0.0)
        nc.vector.memset(ccst[:], 0.0)
        ident = st.tile([128, 128], f16, name="ident_s")
        make_identity(nc, ident[:])
        A_s = st.tile([128, 32, NCP], f8, name="A_s")
        B_s = st.tile([128, 64, NLP], f8, name="B_s")
        for q in range(8):
            nc.gpsimd.dma_start(
                A_s[:, 4 * q : 4 * q + 4, :], A_d[:, 4 * q : 4 * q + 4, :]
            )
            nc.gpsimd.dma_start(
                B_s[:, 8 * q : 8 * q + 8, :], B_d[:, 8 * q : 8 * q + 8, :]
            )

        replica_groups = [list(range(R))]

        def mlp_chunk(pre, rhs, out_tile, out_parts, ncols):
            """3-layer MLP on a [128, ncols] fp16 activation chunk (transposed
            layout). Writes (+bias, no final relu) into out_tile[:out_parts, :ncols]."""
            w1t, w2t, w2t_r = W[pre + "_w1t"], W[pre + "_w2t"], W[pre + "_w2t_r"]
            w3t, w3t_r = W[pre + "_w3t"], W[pre + "_w3t_r"]
            b1, b2, b3 = W[pre + "_b1"], W[pre + "_b2"], W[pre + "_b3"]
            h1 = h1p.tile([128, 4, ncols], f16, tag="h1")
            for c in range(4):
                m = 128 if c < 3 else 16
                ps = ps_net.tile([128, 512], f32, tag="psnet")
                nc.tensor.matmul(
                    ps[:m, :ncols], w1t[:, 128 * c : 128 * c + m], rhs,
                    start=True, stop=True,
                )
                nc.vector.tensor_scalar(
                    h1[:m, c, :], ps[:m, :ncols], b1[:m, c : c + 1], 0.0,
                    AO.add, AO.max,
                )
            h2 = h2p.tile([128, 2, ncols], f16, tag="h2")
            for c, m in ((0, 128), (1, 72)):
                ps = ps_net.tile([128, 512], f32, tag="psnet")
                for k in range(3):
                    nc.tensor.matmul(
                        ps[:m, :ncols],
                        w2t[:, k, 128 * c : 128 * c + m],
                        h1[:, k, :],
                        start=(k == 0),
                        stop=False,
                    )
                nc.tensor.matmul(
                    ps[:m, :ncols],
                    w2t_r[:, 128 * c : 128 * c + m],
                    h1[0:16, 3, :],
                    start=False,
                    stop=True,
                )
                nc.vector.tensor_scalar(
                    h2[:m, c, :], ps[:m, :ncols], b2[:m, c : c + 1], 0.0,
                    AO.add, AO.max,
                )
            o = out_parts
            ps = ps_net.tile([128, 512], f32, tag="psnet")
            nc.tensor.matmul(
                ps[:o, :ncols], w3t[:, :o], h2[:, 0, :], start=True, stop=False
            )
            nc.tensor.matmul(
                ps[:o, :ncols], w3t_r[:, :o], h2[0:72, 1, :], start=False, stop=True
            )
            nc.vector.tensor_scalar(
                out_tile[:o, :ncols], ps[:o, :ncols], b3[:o, 0:1], None, AO.add
            )

        def transpose_to_stage(srcT, ncols, stage, blk0):
            for cb in range(ncols // 128):
                pst = ps_tr.tile([128, 128], f16, tag="pstr")
                nc.tensor.transpose(
                    pst[:], srcT[:, 128 * cb : 128 * (cb + 1)], ident[:]
                )
                nc.vector.tensor_copy(out=stage[:, blk0 + cb, :], in_=pst[:])

        def gate(wih, whh, b, g, msg, h_sl, func, ncols):
            ps = ps_gate.tile([128, 512], f32, tag="gate")
            nc.tensor.matmul(
                ps[:, :ncols], wih[:, 128 * g : 128 * (g + 1)], msg,
                start=True, stop=False,
            )
            nc.tensor.matmul(
                ps[:, :ncols], whh[:, 128 * g : 128 * (g + 1)], h_sl,
                start=False, stop=True,
            )
            t = tmpp.tile([128, ncols], f32, tag="tmp")
            nc.scalar.activation(t[:], ps[:, :ncols], func, bias=b[:, g : g + 1])
            return t

        def lstm_chunk(pre, msg, h_sl, c_sl, ncols):
            """gates order i,f,g,o. h_sl ([128,ncols] f16) and c_sl (f32) updated
            in place."""
            wih, whh, b = W[pre + "_wiht"], W[pre + "_whht"], W[pre + "_b"]
            tf = gate(wih, whh, b, 1, msg, h_sl, AF.Sigmoid, ncols)
            nc.vector.tensor_mul(out=c_sl, in0=tf[:], in1=c_sl)
            ti = gate(wih, whh, b, 0, msg, h_sl, AF.Sigmoid, ncols)
            tg = gate(wih, whh, b, 2, msg, h_sl, AF.Tanh, ncols)
            nc.vector.tensor_mul(out=ti[:], in0=ti[:], in1=tg[:])
            nc.vector.tensor_add(out=c_sl, in0=c_sl, in1=ti[:])
            tc_ = tmpp.tile([128, ncols], f32, tag="tmp")
            nc.scalar.activation(tc_[:], c_sl, AF.Tanh)
            to = gate(wih, whh, b, 3, msg, h_sl, AF.Sigmoid, ncols)
            nc.vector.tensor_mul(out=h_sl, in0=to[:], in1=tc_[:])

        def emit_y_side(ynat_dst):
            """MLP_CL on current ch -> Y -> AllGather -> load into ynat_dst."""
            with nc.named_scope("mlp_cl"):
                yin_t = dramp.tile([NCP, 128], f16, tag="yin")
                yin_v = yin_t.rearrange("(o p) e -> p o e", p=128)
                for n in range(2):
                    yT = xyp.tile([128, 512], f16, tag="xyT")
                    mlp_chunk("cl", ch[:, 512 * n : 512 * (n + 1)], yT, 128, 512)
                    ystage = stp.tile([128, 4, 128], f16, tag="stage")
                    transpose_to_stage(yT, 512, ystage, 0)
                    nc.sync.dma_start(yin_v[:, 4 * n : 4 * n + 4, :], ystage[:, :, :])
            yout_t = dramp.tile([NCP * R, 128], f16, tag="yout", addr_space="Shared")
            nc.gpsimd.collective_compute(
                "AllGather", AO.bypass,
                ins=[yin_t.opt()], outs=[yout_t.opt()],
                replica_groups=replica_groups,
            )
            return yout_t

        def emit_y_load(yout_t, ynat_dst):
            yv = yout_t.rearrange("(r j p) e -> p r j e", p=128, j=8)
            for r in range(R):
                nc.sync.dma_start(ynat_dst[:, 8 * r : 8 * r + 8, :], yv[:, r])

        def emit_x_half(h, xnat_dst):
            """MLP_LC on lh half h -> X half -> AllGather -> load into xnat_dst."""
            with nc.named_scope("mlp_lc"):
                xT = xyp.tile([128, 256], f16, tag="xyT")
                mlp_chunk("lc", lh[:, 256 * h : 256 * (h + 1)], xT, 128, 256)
                xstage = stp.tile([128, 2, 128], f16, tag="xstage")
                transpose_to_stage(xT, 256, xstage, 0)
                xin_t = dramp.tile([NLP // 2, 128], f16, tag=f"xin{h}")
                nc.sync.dma_start(
                    xin_t.rearrange("(o p) e -> p o e", p=128), xstage[:, :, :]
                )
            xout_t = dramp.tile(
                [NLP // 2 * R, 128], f16, tag=f"xout{h}", addr_space="Shared"
            )
            nc.gpsimd.collective_compute(
                "AllGather", AO.bypass,
                ins=[xin_t.opt()], outs=[xout_t.opt()],
                replica_groups=replica_groups,
            )
            # xnat slot order: [h, r, j] — halves contiguous; A is host-permuted
            # to the same row-chunk order.
            return xout_t

        def emit_x_load(h, xout_t, xnat_dst):
            xv = xout_t.rearrange("(q p) e -> p q e", p=128)
            for hq in range(2):
                nc.sync.dma_start(
                    xnat_dst[:, 16 * h + 8 * hq : 16 * h + 8 * hq + 8, :],
                    xv[:, 8 * hq : 8 * hq + 8, :],
                )

        # ---- prologue: X(0), Y(0) ----
        xnat_next = natp.tile([128, 32, 128], f16, tag="xnat")
        ynat_next = natp.tile([128, 64, 128], f16, tag="ynat")
        for h in range(2):
            xo = emit_x_half(h, xnat_next)
            emit_x_load(h, xo, xnat_next)
        yo = emit_y_side(ynat_next)
        emit_y_load(yo, ynat_next)

        for t in range(T):
            xnat_cur, ynat_cur = xnat_next, ynat_next

            # --- dir1(t): LC_msgs^T; LSTM_C(t) ---
            for n in range(2):
                with nc.named_scope("dir1"):
                    psd = ps_dir.tile([128, 512], f32, tag="dir")
                    for k in range(32):
                        nc.tensor.matmul(
                            psd[:],
                            xnat_cur[:, k, :],
                            A_s[:, k, 512 * n : 512 * (n + 1)],
                            start=(k == 0),
                            stop=(k == 31),
                        )
                    msg = msgp.tile([128, 512], f16, tag="msg")
                    nc.vector.tensor_copy(out=msg[:], in_=psd[:])
                with nc.named_scope("lstm_c"):
                    lstm_chunk(
                        "c", msg[:],
                        ch[:, 512 * n : 512 * (n + 1)],
                        ccst[:, 512 * n : 512 * (n + 1)],
                        512,
                    )

            # --- dir2(t) in row halves; LSTM_L(t); MLP_LC(t+1) + AGX(t+1) ---
            if t + 1 < T:
                xnat_next = natp.tile([128, 32, 128], f16, tag="xnat")
            xouts = []
            for h in range(2):
                with nc.named_scope("dir2"):
                    psd2 = ps_dir.tile([128, 512], f32, tag="dir")
                    for k in range(64):
                        nc.tensor.matmul(
                            psd2[:, :256],
                            ynat_cur[:, k, :],
                            B_s[:, k, 256 * h : 256 * (h + 1)],
                            start=(k == 0),
                            stop=(k == 63),
                        )
                    msgl = msgp.tile([128, 256], f16, tag="msg")
                    nc.vector.tensor_copy(out=msgl[:], in_=psd2[:, :256])
                with nc.named_scope("lstm_l"):
                    lstm_chunk(
                        "l", msgl[:],
                        lh[:, 256 * h : 256 * (h + 1)],
                        lcst[:, 256 * h : 256 * (h + 1)],
                        256,
                    )
                if t + 1 < T:
                    xouts.append(emit_x_half(h, xnat_next))

            # --- MLP_CL(t+1) + AGY(t+1): PE filler while AGX(t+1) flies;
            #     AGY runs on CC after the AGX halves, during dir1(t+1) ---
            if t + 1 < T:
                ynat_next = natp.tile([128, 64, 128], f16, tag="ynat")
                yo = emit_y_side(ynat_next)
                for h in range(2):
                    emit_x_load(h, xouts[h], xnat_next)
                emit_y_load(yo, ynat_next)

        # --- final: vote MLP on Lh ---
        votes_sb = st.tile([1, 512], f32, name="votes_sb")
        mlp_chunk("v", lh[:, :], votes_sb, 1, 512)
        nc.sync.dma_start(votes_d[:], votes_sb[0:1, :])

    nc.compile()
    return nc


def prep_inputs(inputs):
    """Host-side: shard + lay out all tensors per core in final dtypes."""
    import ml_dtypes

    f8 = ml_dtypes.float8_e4m3
    f16 = np.float16
    f32 = np.float32
    M = np.asarray(inputs["M"], f32)

    MP = np.zeros((NLP * R, NCP * R), f32)
    for rl in range(R):
        for rc in range(R):
            MP[NLP * rl : NLP * rl + NLR, NCP * rc : NCP * rc + NCR] = M[
                NLR * rl : NLR * (rl + 1), NCR * rc : NCR * (rc + 1)
            ]

    shared = {}

    def mlp_prep(pre, W1, b1, W2, b2, W3, b3):
        wout = W3.shape[0]
        shared[pre + "_w1t"] = np.ascontiguousarray(np.asarray(W1, f32).T, f16)
        w2t_full = np.zeros((512, 200), f32)
        w2t_full[:400] = np.asarray(W2, f32).T
        shared[pre + "_w2t"] = np.ascontiguousarray(
            w2t_full[:384].reshape(3, 128, 200).transpose(1, 0, 2), f16
        )
        shared[pre + "_w2t_r"] = np.ascontiguousarray(w2t_full[384:400], f16)
        w3t = np.ascontiguousarray(np.asarray(W3, f32).T, f32)   # [200, wout]
        shared[pre + "_w3t"] = np.ascontiguousarray(w3t[:128], f16)
        shared[pre + "_w3t_r"] = np.ascontiguousarray(w3t[128:200], f16)
        b1p = np.zeros((512,), f32); b1p[:400] = b1
        shared[pre + "_b1"] = np.ascontiguousarray(b1p.reshape(4, 128).T, f32)
        b2p = np.zeros((256,), f32); b2p[:200] = b2
        shared[pre + "_b2"] = np.ascontiguousarray(b2p.reshape(2, 128).T, f32)
        b3p = np.zeros((wout, 1), f32); b3p[:, 0] = b3
        shared[pre + "_b3"] = b3p

    mlp_prep("lc", inputs["LC_W1"], inputs["LC_b1"], inputs["LC_W2"],
             inputs["LC_b2"], inputs["LC_W3"], inputs["LC_b3"])
    mlp_prep("cl", inputs["CL_W1"], inputs["CL_b1"], inputs["CL_W2"],
             inputs["CL_b2"], inputs["CL_W3"], inputs["CL_b3"])
    mlp_prep("v", inputs["V_W1"], inputs["V_b1"], inputs["V_W2"],
             inputs["V_b2"], inputs["V_W3"], inputs["V_b3"])

    for pre, Wih, Whh, b in (
        ("l", inputs["L_Wih"], inputs["L_Whh"], inputs["L_b"]),
        ("c", inputs["C_Wih"], inputs["C_Whh"], inputs["C_b"]),
    ):
        shared[pre + "_wiht"] = np.ascontiguousarray(np.asarray(Wih, f32).T, f16)
        shared[pre + "_whht"] = np.ascontiguousarray(np.asarray(Whh, f32).T, f16)
        shared[pre + "_b"] = np.ascontiguousarray(
            np.asarray(b, f32).reshape(4, 128).T, f32
        )

    shared["lh0"] = np.ascontiguousarray(
        np.repeat(np.asarray(inputs["L_init"], f32)[:, None], NLP, 1), f16
    )
    shared["ch0"] = np.ascontiguousarray(
        np.repeat(np.asarray(inputs["C_init"], f32)[:, None], NCP, 1), f16
    )

    in_maps = []
    for r in range(R):
        A_r = MP[:, NCP * r : NCP * (r + 1)]                       # [4096, 1024]
        B_r = MP[NLP * r : NLP * (r + 1), :].T                     # [8192, 512]
        m = dict(shared)
        # row-chunk order must match xnat slot order o' = 16h + 2r + j
        # (lit rows for slot o' are 512r + 256h + 128j .. +128)
        A_chunks = A_r.reshape(32, 128, NCP)
        perm = [4 * rr + 2 * hh + jj
                for hh in range(2) for rr in range(8) for jj in range(2)]
        m["A"] = np.ascontiguousarray(
            A_chunks[perm].transpose(1, 0, 2), f8
        )
        m["B"] = np.ascontiguousarray(
            B_r.reshape(64, 128, NLP).transpose(1, 0, 2), f8
        )
        in_maps.append(m)
    return in_maps


def _install_ntff_hook():
    """The image's antenv package lacks axon_hooks; shim it so
    run_bass_kernel_spmd(trace=True) can capture NTFF profiles."""
    import types

    if "antenv.axon_hooks" in sys.modules:
        return
    if "/root/.axon_site" not in sys.path:
        sys.path.insert(0, "/root/.axon_site")
    try:
        from trn_agent_boot.trn_boot import _ntff_profile_via_ctypes

        hook = _ntff_profile_via_ctypes("/opt/axon/libaxon_pjrt.so")
    except Exception:
        hook = None
    mod = types.ModuleType("antenv.axon_hooks")
    state = {"hook": hook}
    mod.set_axon_ntff_profile_hook = lambda h: state.__setitem__("hook", h)
    mod.get_axon_ntff_profile_hook = lambda: state["hook"]
    sys.modules["antenv.axon_hooks"] = mod
    try:
        import antenv

        antenv.axon_hooks = mod
    except Exception:
        pass


def kernel(**inputs):
    global LAST_EXEC_NS, LAST_RESULTS
    from concourse.bass_utils import run_bass_kernel_spmd

    T = T_STEPS
    if T not in _program_cache:
        _program_cache[T] = build_program(T)
    nc = _program_cache[T]

    in_maps = prep_inputs(inputs)
    trace = os.environ.get("NSAT_TRACE", "0") == "1"
    if trace:
        _install_ntff_hook()
    res = run_bass_kernel_spmd(nc, in_maps, list(range(R)), trace=trace)
    LAST_EXEC_NS = res.exec_time_ns
    LAST_RESULTS = res
    votes_pre = np.concatenate(
        [res.results[r]["votes"][0, :NLR].astype(np.float64) for r in range(R)]
    )
    avg = np.mean(1.0 / (1.0 + np.exp(-votes_pre)))
    return np.asarray(np.log(avg / (1.0 - avg)), dtype=np.float32)
